# revision 18
# baseline (speedup 1.0000x reference)
"""MoE routing kernel for 8 Trainium2 NeuronCores (Bass/Tile, SPMD).

Strategy (expert-parallel, per the sharding hint):
  - Host computes the gate (softmax + top-2) and dispatches tokens: each of
    the 8 cores owns 2 of the 16 routed experts and receives only the tokens
    routed to its experts.  Experts are sorted by token count and paired
    (rank i with rank 15-i) so per-slot capacities (cap0 = max big-slot
    count, cap1 = max small-slot count) carry minimal padding.
  - The output layer (ow) commutes with the weighted combine, so it is
    folded into each expert's second matmul on the host (w2[e].T @ ow.T),
    shrinking stage-2 work 4x.  Bias terms that commute (b2, sb2, ob) are
    applied analytically on the host.
  - The shared expert is sharded over its intermediate dim (2048/8 = 256
    rows per core); every core computes a partial over all 2048 tokens.
  - Device-side scheduling: weights are packed in consumption order and
    DMAed in m-tile-group granularity on the two HWDGE queues (SP = expert
    weights, ACT = activations/shared) so the in-order PE stream starts
    ~3us in and pipelines with the weight stream.  A short dummy-matmul
    warmup flips the PE HAM clock gate to 2.4 GHz during the DMA ramp.
    Element-wise work is spread over ACT (lrelu+bias), DVE (bias add) and
    GpSimd (multiply).  Outputs stream out incrementally in fp16 on the
    SWDGE queue.
  - Host combines: scatter-add of combine-weight-scaled routed partials +
    shared partials + analytic bias terms.
"""
import sys

if "/opt/trn_rl_repo" not in sys.path:
    sys.path.insert(0, "/opt/trn_rl_repo")

import os
import numpy as np
import concourse.bass as bass
import concourse.tile as tile
from concourse import mybir
from concourse.bass_utils import run_bass_kernel_spmd

B = 2048
W = 512
E = 16
TOPK = 2
INTER = 1024
SH = 2048
OUT = 128
NCORES = 8
EPC = E // NCORES          # experts per core = 2
SHS = SH // NCORES         # shared-expert inter slice per core = 256
KW = W // 128              # k-tiles over W = 4
MI = INTER // 128          # m-tiles over INTER = 8
MS = SHS // 128            # m-tiles over shared slice = 2
MBLK = 2 * KW * 128 + OUT        # columns per m-tile weight block: w1|w3|w2ot
GCOLS = 8 * MBLK                 # columns per expert (8 m-tile blocks)
F32 = mybir.dt.float32
F16 = mybir.dt.float16
DT = F16                   # device datapath dtype for matmul operands
NPDT = np.float16

# set by test.py to collect a profile; results stashed in LAST_RESULTS
TRACE = False
TRACE_KW = {}
LAST_RESULTS = None


def _legalize_waits(nc):
    """This container's walrus accepts at most 1 sync wait per instruction
    (2 for EventSemaphore).  Hoist excess waits emitted by the Tile
    scheduler into standalone EventSemaphore instructions."""
    for fn in nc.m.functions:
        for blk in fn.blocks:
            out = []
            changed = False
            for inst in blk.instructions:
                si = getattr(inst, "sync_info", None)
                waits = list(si.on_wait) if si is not None and si.on_wait else []
                cap = 2 if isinstance(inst, mybir.InstEventSemaphore) else 1
                if len(waits) > cap:
                    extra, keep = waits[:-cap], waits[-cap:]
                    for i in range(0, len(extra), 2):
                        out.append(mybir.InstEventSemaphore(
                            name=nc.get_next_instruction_name(),
                            engine=inst.engine,
                            ins=[], outs=[],
                            sync_info=mybir.SyncInfo(
                                on_wait=list(extra[i:i + 2]), on_update=[]),
                        ))
                    si.on_wait = keep
                    changed = True
                out.append(inst)
            if changed:
                blk.instructions = out
    return nc


def _build_nc(cap0, cap1, legalize=True):
    """SPMD Bass program for per-slot token capacities (cap0, cap1)."""
    nc = bass.Bass("TRN2", target_bir_lowering=False, debug=False)
    caps = (cap0, cap1)

    def din(name, f, dt=DT):
        return nc.dram_tensor(name, [128, f], dt, kind="ExternalInput").ap()

    NCH = 8                         # shared-expert token chunks
    CH = B // NCH                   # tokens per chunk = 256
    xga = din("xga", KW * cap0)     # gathered tokens, slot A, k-major
    xgb = din("xgb", KW * cap1)
    wga = din("wga", GCOLS)         # slot A weights: 8 m-blocks [w1|w3|w2ot]
    wgb = din("wgb", GCOLS)
    swp = din("swp", MS * MBLK)     # shared weights: 2 m-blocks
    xt = din("xt", KW * B)          # x.T in NCH chunk-major blocks of [128, KW*CH]
    bias = din("bias", 4 * MI + 2 * MS, F32)

    yr = nc.dram_tensor("yr", [128, cap0 + cap1], DT, kind="ExternalOutput").ap()
    zt = nc.dram_tensor("zt", [128, B], DT, kind="ExternalOutput").ap()

    LR = mybir.ActivationFunctionType.Lrelu

    with tile.TileContext(nc) as tc:
        with tc.tile_pool(name="wts", bufs=1) as wts, \
             tc.tile_pool(name="work", bufs=3) as work, \
             tc.tile_pool(name="hts", bufs=1) as hts, \
             tc.tile_pool(name="outs", bufs=2) as outs, \
             tc.tile_pool(name="ps", bufs=2, space="PSUM") as ps:

            # ---- PE warmup tile (zeroed by Pool engine; no DMA dependency)
            warm = wts.tile([128, 512], DT, tag="warm")
            nc.gpsimd.memset(warm[:], 0.0)

            # ---- input DMAs, consumption-ordered on the two HWDGE queues
            bias_t = wts.tile([128, bias.shape[1]], F32, tag="bias")
            xga_t = wts.tile([128, KW * cap0], DT, tag="xga")
            xgb_t = wts.tile([128, KW * cap1], DT, tag="xgb")
            swp_t = wts.tile([128, MS * MBLK], DT, tag="swp")
            xt_ts = [wts.tile([128, KW * CH], DT, tag=f"xt{c}", name=f"xt{c}")
                     for c in range(NCH)]
            wga_t = wts.tile([128, GCOLS], DT, tag="wga")
            wgb_t = wts.tile([128, GCOLS], DT, tag="wgb")

            def xt_dma(eng, c):
                eng.dma_start(xt_ts[c][:], xt[:, c * KW * CH:(c + 1) * KW * CH])

            def wg_dma(eng, wt, wg, m0, m1):
                eng.dma_start(wt[:, m0 * MBLK:m1 * MBLK], wg[:, m0 * MBLK:m1 * MBLK])

            # Both queues feed the first work units up front (shared chunk 0
            # then expert-A m0), then alternate xt chunks with weight groups
            # in consumption order.  ~3.6MB per queue.
            # ACT queue:
            nc.scalar.dma_start(bias_t[:], bias[:])
            nc.scalar.dma_start(swp_t[:, :MBLK], swp[:, :MBLK])
            nc.scalar.dma_start(swp_t[:, MBLK:], swp[:, MBLK:])
            xt_dma(nc.scalar, 1)
            xt_dma(nc.scalar, 3)
            nc.scalar.dma_start(xgb_t[:], xgb[:])
            xt_dma(nc.scalar, 5)
            wg_dma(nc.scalar, wgb_t, wgb, 0, 2)
            xt_dma(nc.scalar, 7)
            wg_dma(nc.scalar, wgb_t, wgb, 2, 4)
            wg_dma(nc.scalar, wgb_t, wgb, 4, 6)
            wg_dma(nc.scalar, wgb_t, wgb, 6, 8)
            # SP queue:
            xt_dma(nc.sync, 0)
            wg_dma(nc.sync, wga_t, wga, 0, 1)
            for k in range(KW):
                nc.sync.dma_start(xga_t[:, k * cap0:(k + 1) * cap0],
                                  xga[:, k * cap0:(k + 1) * cap0])
            wg_dma(nc.sync, wga_t, wga, 1, 2)
            xt_dma(nc.sync, 2)
            wg_dma(nc.sync, wga_t, wga, 2, 4)
            xt_dma(nc.sync, 4)
            wg_dma(nc.sync, wga_t, wga, 4, 6)
            xt_dma(nc.sync, 6)
            wg_dma(nc.sync, wga_t, wga, 6, 8)

            # ---- PE warmup: one accumulation group of cold N=512 matmuls
            # bridges the DMA ramp and flips the HAM clock gate to 8/8 right
            # as real work arrives.
            NWARM = 9
            pw = ps.tile([128, 512], F32, tag="warm", bufs=1)
            for i in range(NWARM):
                nc.tensor.matmul(pw[:], warm[:, 0:128], warm[:],
                                 start=(i == 0), stop=(i == NWARM - 1))

            def b_ap(col):  # [128,1] per-partition bias column
                return bias_t[:, col:col + 1]

            LAG = 2

            def expert_slot(slot):
                """Generator: one step per m-tile, stage-2 trails by LAG."""
                cap = caps[slot]
                xg_t = (xga_t, xgb_t)[slot]
                wg_t = (wga_t, wgb_t)[slot]
                boff = slot * 2 * MI
                py = ps.tile([128, cap], F32, tag="py", bufs=1)
                pend = []

                def stage2(m, ht):
                    lhs = wg_t[:, m * MBLK + KW * 256:m * MBLK + KW * 256 + OUT]
                    nc.tensor.matmul(py[:], lhs, ht[:],
                                     start=(m == 0), stop=(m == MI - 1))

                for m in range(MI):
                    mb = m * MBLK
                    p1 = ps.tile([128, cap], F32, tag="p1", bufs=2)
                    p3 = ps.tile([128, cap], F32, tag="p3", bufs=2)
                    for k in range(KW):
                        lhs = wg_t[:, mb + k * 128:mb + (k + 1) * 128]
                        rhs = xg_t[:, k * cap:(k + 1) * cap]
                        nc.tensor.matmul(p1[:], lhs, rhs, start=(k == 0), stop=(k == KW - 1))
                    for k in range(KW):
                        lhs = wg_t[:, mb + KW * 128 + k * 128:mb + KW * 128 + (k + 1) * 128]
                        rhs = xg_t[:, k * cap:(k + 1) * cap]
                        nc.tensor.matmul(p3[:], lhs, rhs, start=(k == 0), stop=(k == KW - 1))
                    a = work.tile([128, cap], DT, tag="act_a")
                    nc.scalar.activation(a[:], p1[:], LR, bias=b_ap(boff + m), alpha=0.01)
                    t3 = work.tile([128, cap], DT, tag="act_b")
                    nc.vector.tensor_scalar_add(t3[:], p3[:], b_ap(boff + MI + m))
                    ht = hts.tile([128, cap], DT, tag="ht", bufs=LAG + 3)
                    nc.gpsimd.tensor_mul(ht[:], a[:], t3[:])
                    pend.append((m, ht))
                    if len(pend) > LAG:
                        stage2(*pend.pop(0))
                    if m != MI - 1:
                        yield
                for args in pend:
                    stage2(*args)
                yo = outs.tile([128, cap], DT, tag="yo")
                nc.vector.tensor_copy(yo[:], py[:])
                off = 0 if slot == 0 else cap0
                nc.sync.dma_start(yr[:, off:off + cap], yo[:])
                yield

            def shared_steps():
                """Generator: one step per (chunk, m-tile); stage-2 flush of
                chunk c happens during chunk c+1 (full-chunk lag)."""
                pend = []

                def flush(pz, c, hs_list):
                    for m, hs in hs_list:
                        lhs = swp_t[:, m * MBLK + KW * 256:m * MBLK + KW * 256 + OUT]
                        nc.tensor.matmul(pz[:], lhs, hs[:], start=(m == 0), stop=(m == MS - 1))
                    zo = outs.tile([128, CH], DT, tag="zo")
                    nc.vector.tensor_copy(zo[:], pz[:])
                    nc.sync.dma_start(zt[:, c * CH:(c + 1) * CH], zo[:])

                for c in range(NCH):
                    pz = ps.tile([128, CH], F32, tag="pz", bufs=1)
                    hs_list = []
                    for m in range(MS):
                        mb = m * MBLK
                        p1 = ps.tile([128, CH], F32, tag="p1", bufs=2)
                        p3 = ps.tile([128, CH], F32, tag="p3", bufs=2)
                        for k in range(KW):
                            lhs = swp_t[:, mb + k * 128:mb + (k + 1) * 128]
                            rhs = xt_ts[c][:, k * CH:(k + 1) * CH]
                            nc.tensor.matmul(p1[:], lhs, rhs, start=(k == 0), stop=(k == KW - 1))
                        for k in range(KW):
                            lhs = swp_t[:, mb + KW * 128 + k * 128:mb + KW * 128 + (k + 1) * 128]
                            rhs = xt_ts[c][:, k * CH:(k + 1) * CH]
                            nc.tensor.matmul(p3[:], lhs, rhs, start=(k == 0), stop=(k == KW - 1))
                        a = work.tile([128, CH], DT, tag="act_a")
                        nc.scalar.activation(a[:], p1[:], LR,
                                             bias=b_ap(4 * MI + m), alpha=0.01)
                        t3 = work.tile([128, CH], DT, tag="act_b")
                        nc.vector.tensor_scalar_add(t3[:], p3[:], b_ap(4 * MI + MS + m))
                        hs = hts.tile([128, CH], DT, tag="hs", bufs=MS + 2)
                        nc.gpsimd.tensor_mul(hs[:], a[:], t3[:])
                        hs_list.append((m, hs))
                        if not (c == NCH - 1 and m == MS - 1):
                            yield
                    pend.append((pz, c, hs_list))
                    if len(pend) > 1:
                        flush(*pend.pop(0))
                for args in pend:
                    flush(*args)
                yield

            # ---- interleave: A/B = expert slot m-tile, S = shared m-tile.
            # Default tuned to the DMA arrival schedule (SP ~0.58MB/group,
            # ACT xga+swp+xt chunks); override via K_PATTERN for experiments.
            pattern = os.environ.get(
                "K_PATTERN",
                "SS AA SS AA SS AA SS AA SS BB SS BB SS BB SS BB")
            gens = {"A": expert_slot(0), "B": expert_slot(1), "S": shared_steps()}
            for ch in pattern:
                if ch == " ":
                    continue
                g = gens.get(ch)
                if g is None:
                    continue
                try:
                    next(g)
                except StopIteration:
                    gens[ch] = None
            for g in gens.values():
                if g is None:
                    continue
                for _ in g:
                    pass

    if legalize:
        _legalize_waits(nc)
    return nc


_NC_CACHE = {}


def _kblocks(mat, nk):
    """[nk*128, F] -> [128, nk*F], col block k = mat[128k:128(k+1), :]."""
    f = mat.shape[1]
    return mat.reshape(nk, 128, f).transpose(1, 0, 2).reshape(128, -1)


def _pack_mblocks(w1e, w3e, w2oe, nm):
    """Consumption-ordered weight pack: nm m-blocks of
    [w1 k-blocks (KW*128) | w3 k-blocks | w2ot (OUT)] = MBLK cols each."""
    # w1e/w3e: [nm*128, W];  w2oe: [nm*128, OUT]
    w1b = w1e.T.reshape(KW, 128, nm, 128)   # [k, kr, m, mc]
    w3b = w3e.T.reshape(KW, 128, nm, 128)
    w2b = w2oe.reshape(nm, 128, OUT)        # [m, mr, OUT]
    cols = []
    for m in range(nm):
        for k in range(KW):
            cols.append(w1b[k, :, m, :])
        for k in range(KW):
            cols.append(w3b[k, :, m, :])
        cols.append(w2b[m])
    return np.concatenate(cols, axis=1).astype(NPDT)


def prepare(x, task_id, gate_w, w1, b1, w2, b2, w3, b3,
            sw1, sb1, sw2, sb2, sw3, sb3, ow, ob):
    """Host-side routing + packing."""
    x = np.asarray(x, np.float32)
    f32 = lambda a: np.asarray(a, np.float32)
    gate_w, w1, b1, w2, b2, w3, b3 = map(f32, (gate_w, w1, b1, w2, b2, w3, b3))
    sw1, sb1, sw2, sb2, sw3, sb3, ow, ob = map(f32, (sw1, sb1, sw2, sb2, sw3, sb3, ow, ob))

    # ---- host gate: softmax + top-2 ----
    logits = x @ gate_w.T
    logits -= logits.max(axis=1, keepdims=True)
    ex = np.exp(logits)
    scores = ex / ex.sum(axis=1, keepdims=True)
    order = np.argsort(-scores, axis=1, kind="stable")[:, :TOPK]

    tok_lists = [np.nonzero((order == e).any(axis=1))[0] for e in range(E)]
    counts = np.array([len(t) for t in tok_lists])
    rank = np.argsort(-counts, kind="stable")
    slotA = [int(rank[i]) for i in range(NCORES)]          # big experts
    slotB = [int(rank[E - 1 - i]) for i in range(NCORES)]  # small experts
    r16 = lambda n: max(64, -(-n // 16) * 16)
    cap0 = r16(max(counts[e] for e in slotA))
    cap1 = r16(max(counts[e] for e in slotB))

    key = (cap0, cap1)
    if key not in _NC_CACHE:
        _NC_CACHE[key] = _build_nc(cap0, cap1)
    nc = _NC_CACHE[key]

    # ---- shared-expert packing (same for every core except slice) ----
    xtp = x.T.reshape(KW, 128, 8, 256).transpose(1, 2, 0, 3).reshape(128, -1)
    xtp = np.ascontiguousarray(xtp).astype(NPDT)   # chunk-major, k inside

    in_maps = []
    for c in range(NCORES):
        eA, eB = slotA[c], slotB[c]
        s = slice(c * SHS, (c + 1) * SHS)

        def gather(e, cap):
            toks = tok_lists[e]
            xg = np.zeros((W, cap), np.float32)
            xg[:, :len(toks)] = x[toks].T
            return _kblocks(xg, KW).astype(NPDT)

        swp = _pack_mblocks(sw1[s], sw3[s], sw2[:, s].T @ ow.T, MS)

        bias_cols = [b1[eA].reshape(MI, 128).T, b3[eA].reshape(MI, 128).T,
                     b1[eB].reshape(MI, 128).T, b3[eB].reshape(MI, 128).T,
                     sb1[s].reshape(MS, 128).T, sb3[s].reshape(MS, 128).T]

        in_maps.append({
            "xga": gather(eA, cap0),
            "xgb": gather(eB, cap1),
            "wga": _pack_mblocks(w1[eA], w3[eA], w2[eA].T @ ow.T, MI),
            "wgb": _pack_mblocks(w1[eB], w3[eB], w2[eB].T @ ow.T, MI),
            "swp": swp,
            "xt": xtp,
            "bias": np.ascontiguousarray(np.concatenate(bias_cols, axis=1)),
        })

    combine_w = np.zeros((B, E), np.float32)
    rows = np.arange(B)
    combine_w[rows[:, None], order] = np.take_along_axis(scores, order, axis=1)
    base = combine_w @ (b2 @ ow.T) + sb2 @ ow.T + ob

    return dict(nc=nc, in_maps=in_maps, cap0=cap0, cap1=cap1,
                slotA=slotA, slotB=slotB, tok_lists=tok_lists,
                combine_w=combine_w, base=base)


def combine(p, results):
    """Combine per-core device partials into the full [B, OUT] output."""
    cap0, tok_lists, combine_w = p["cap0"], p["tok_lists"], p["combine_w"]
    out = p["base"].astype(np.float32).copy()
    for c in range(NCORES):
        r = results[c]
        out += r["zt"].astype(np.float32).T
        for slot, e in ((0, p["slotA"][c]), (1, p["slotB"][c])):
            toks = tok_lists[e]
            off = 0 if slot == 0 else cap0
            yre = r["yr"][:, off:off + len(toks)].astype(np.float32)
            out[toks] += combine_w[toks, e][:, None] * yre.T
    return out


def kernel(x, task_id, gate_w, w1, b1, w2, b2, w3, b3,
           sw1, sb1, sw2, sb2, sw3, sb3, ow, ob):
    global LAST_RESULTS
    p = prepare(x, task_id, gate_w, w1, b1, w2, b2, w3, b3,
                sw1, sb1, sw2, sb2, sw3, sb3, ow, ob)
    res = run_bass_kernel_spmd(
        p["nc"], p["in_maps"], core_ids=list(range(NCORES)),
        trace=TRACE, **TRACE_KW)
    LAST_RESULTS = res
    return combine(p, res.results)


# revision 22
# speedup vs baseline: 1.0282x; 1.0282x over previous
"""MoE routing kernel for 8 Trainium2 NeuronCores (Bass/Tile, SPMD).

Strategy (expert-parallel, per the sharding hint):
  - Host computes the gate (softmax + top-2) and dispatches tokens: each of
    the 8 cores owns 2 of the 16 routed experts and receives only the tokens
    routed to its experts.  Experts are sorted by token count and paired
    (rank i with rank 15-i) so per-slot capacities (cap0 = max big-slot
    count, cap1 = max small-slot count) carry minimal padding.
  - The output layer (ow) commutes with the weighted combine, so it is
    folded into each expert's second matmul on the host (w2[e].T @ ow.T),
    shrinking stage-2 work 4x.  Bias terms that commute (b2, sb2, ob) are
    applied analytically on the host.
  - The shared expert is sharded over its intermediate dim (2048/8 = 256
    rows per core); every core computes a partial over all 2048 tokens.
  - Device-side scheduling: weights are packed in consumption order and
    DMAed in m-tile-group granularity on the two HWDGE queues (SP = expert
    weights, ACT = activations/shared) so the in-order PE stream starts
    ~3us in and pipelines with the weight stream.  A short dummy-matmul
    warmup flips the PE HAM clock gate to 2.4 GHz during the DMA ramp.
    Element-wise work is spread over ACT (lrelu+bias), DVE (bias add) and
    GpSimd (multiply).  Outputs stream out incrementally in fp16 on the
    SWDGE queue.
  - Host combines: scatter-add of combine-weight-scaled routed partials +
    shared partials + analytic bias terms.
"""
import sys

if "/opt/trn_rl_repo" not in sys.path:
    sys.path.insert(0, "/opt/trn_rl_repo")

import os
import numpy as np
import concourse.bass as bass
import concourse.tile as tile
from concourse import mybir
from concourse.bass_utils import run_bass_kernel_spmd

B = 2048
W = 512
E = 16
TOPK = 2
INTER = 1024
SH = 2048
OUT = 128
NCORES = 8
EPC = E // NCORES          # experts per core = 2
SHS = SH // NCORES         # shared-expert inter slice per core = 256
KW = W // 128              # k-tiles over W = 4
MI = INTER // 128          # m-tiles over INTER = 8
MS = SHS // 128            # m-tiles over shared slice = 2
MBLK = 2 * KW * 128 + OUT        # columns per m-tile weight block: w1|w3|w2ot
GCOLS = 8 * MBLK                 # columns per expert (8 m-tile blocks)
F32 = mybir.dt.float32
F16 = mybir.dt.float16
DT = F16                   # device datapath dtype for matmul operands
NPDT = np.float16

# set by test.py to collect a profile; results stashed in LAST_RESULTS
TRACE = False
TRACE_KW = {}
LAST_RESULTS = None


def _legalize_waits(nc):
    """This container's walrus accepts at most 1 sync wait per instruction
    (2 for EventSemaphore).  Hoist excess waits emitted by the Tile
    scheduler into standalone EventSemaphore instructions."""
    for fn in nc.m.functions:
        for blk in fn.blocks:
            out = []
            changed = False
            for inst in blk.instructions:
                si = getattr(inst, "sync_info", None)
                waits = list(si.on_wait) if si is not None and si.on_wait else []
                cap = 2 if isinstance(inst, mybir.InstEventSemaphore) else 1
                if len(waits) > cap:
                    extra, keep = waits[:-cap], waits[-cap:]
                    for i in range(0, len(extra), 2):
                        out.append(mybir.InstEventSemaphore(
                            name=nc.get_next_instruction_name(),
                            engine=inst.engine,
                            ins=[], outs=[],
                            sync_info=mybir.SyncInfo(
                                on_wait=list(extra[i:i + 2]), on_update=[]),
                        ))
                    si.on_wait = keep
                    changed = True
                out.append(inst)
            if changed:
                blk.instructions = out
    return nc


def _build_nc(cap0, cap1, legalize=True):
    """SPMD Bass program for per-slot token capacities (cap0, cap1)."""
    nc = bass.Bass("TRN2", target_bir_lowering=False, debug=False)
    caps = (cap0, cap1)

    def din(name, f, dt=DT):
        return nc.dram_tensor(name, [128, f], dt, kind="ExternalInput").ap()

    NCH = 8                         # shared-expert token chunks
    CH = B // NCH                   # tokens per chunk = 256
    xga = din("xga", KW * cap0)     # gathered tokens, slot A, k-major
    xgb = din("xgb", KW * cap1)
    wga = din("wga", GCOLS)         # slot A weights: 8 m-blocks [w1|w3|w2ot]
    wgb = din("wgb", GCOLS)
    swp = din("swp", MS * MBLK)     # shared weights: 2 m-blocks
    xt = din("xt", KW * B)          # x.T in NCH chunk-major blocks of [128, KW*CH]
    bias = din("bias", 4 * MI + 2 * MS, F32)

    yr = nc.dram_tensor("yr", [128, cap0 + cap1], DT, kind="ExternalOutput").ap()
    zt = nc.dram_tensor("zt", [128, B], DT, kind="ExternalOutput").ap()

    LR = mybir.ActivationFunctionType.Lrelu

    with tile.TileContext(nc) as tc:
        with tc.tile_pool(name="wts", bufs=1) as wts, \
             tc.tile_pool(name="work", bufs=3) as work, \
             tc.tile_pool(name="hts", bufs=1) as hts, \
             tc.tile_pool(name="outs", bufs=2) as outs, \
             tc.tile_pool(name="ps", bufs=2, space="PSUM") as ps:

            # ---- PE warmup tile (zeroed by Pool engine; no DMA dependency)
            warm = wts.tile([128, 512], DT, tag="warm")
            nc.gpsimd.memset(warm[:], 0.0)

            # ---- input DMAs, consumption-ordered on the two HWDGE queues
            bias_t = wts.tile([128, bias.shape[1]], F32, tag="bias")
            xga_t = wts.tile([128, KW * cap0], DT, tag="xga")
            xgb_t = wts.tile([128, KW * cap1], DT, tag="xgb")
            swp_t = wts.tile([128, MS * MBLK], DT, tag="swp")
            xt_ts = [wts.tile([128, KW * CH], DT, tag=f"xt{c}", name=f"xt{c}")
                     for c in range(NCH)]
            wga_t = wts.tile([128, GCOLS], DT, tag="wga")
            wgb_t = wts.tile([128, GCOLS], DT, tag="wgb")

            def xt_dma(eng, c):
                eng.dma_start(xt_ts[c][:], xt[:, c * KW * CH:(c + 1) * KW * CH])

            def wg_dma(eng, wt, wg, m0, m1):
                eng.dma_start(wt[:, m0 * MBLK:m1 * MBLK], wg[:, m0 * MBLK:m1 * MBLK])

            # Both queues front-load the shared-expert chunk-0 work (most
            # PE-work per byte), then stream weights in consumption order.
            # SP queue (7 triggers, ~3.1MB):
            xt_dma(nc.sync, 0)
            xt_dma(nc.sync, 1)
            nc.sync.dma_start(xga_t[:], xga[:])
            for g in range(4):
                wg_dma(nc.sync, wga_t, wga, 2 * g, 2 * g + 2)
            # ACT queue (13 triggers, ~4.8MB):
            nc.scalar.dma_start(bias_t[:], bias[:])
            nc.scalar.dma_start(swp_t[:, :MBLK], swp[:, :MBLK])
            nc.scalar.dma_start(swp_t[:, MBLK:], swp[:, MBLK:])
            xt_dma(nc.scalar, 2)
            xt_dma(nc.scalar, 3)
            nc.scalar.dma_start(xgb_t[:], xgb[:])
            xt_dma(nc.scalar, 4)
            wg_dma(nc.scalar, wgb_t, wgb, 0, 2)
            xt_dma(nc.scalar, 5)
            xt_dma(nc.scalar, 6)
            wg_dma(nc.scalar, wgb_t, wgb, 2, 4)
            xt_dma(nc.scalar, 7)
            wg_dma(nc.scalar, wgb_t, wgb, 4, 8)

            # ---- PE warmup: one accumulation group of cold N=512 matmuls
            # bridges the DMA ramp and flips the HAM clock gate to 8/8 right
            # as real work arrives.
            NWARM = 8
            pw = ps.tile([128, 512], F32, tag="warm", bufs=1)
            for i in range(NWARM):
                nc.tensor.matmul(pw[:], warm[:, 0:128], warm[:],
                                 start=(i == 0), stop=(i == NWARM - 1))

            def b_ap(col):  # [128,1] per-partition bias column
                return bias_t[:, col:col + 1]

            LAG = 2

            def expert_slot(slot):
                """Generator: one step per m-tile, stage-2 trails by LAG."""
                cap = caps[slot]
                xg_t = (xga_t, xgb_t)[slot]
                wg_t = (wga_t, wgb_t)[slot]
                boff = slot * 2 * MI
                py = ps.tile([128, cap], F32, tag="py", bufs=1)
                pend = []

                def stage2(m, ht):
                    lhs = wg_t[:, m * MBLK + KW * 256:m * MBLK + KW * 256 + OUT]
                    nc.tensor.matmul(py[:], lhs, ht[:],
                                     start=(m == 0), stop=(m == MI - 1))

                for m in range(MI):
                    mb = m * MBLK
                    p1 = ps.tile([128, cap], F32, tag="p1", bufs=2)
                    p3 = ps.tile([128, cap], F32, tag="p3", bufs=2)
                    for k in range(KW):
                        lhs = wg_t[:, mb + k * 128:mb + (k + 1) * 128]
                        rhs = xg_t[:, k * cap:(k + 1) * cap]
                        nc.tensor.matmul(p1[:], lhs, rhs, start=(k == 0), stop=(k == KW - 1))
                    for k in range(KW):
                        lhs = wg_t[:, mb + KW * 128 + k * 128:mb + KW * 128 + (k + 1) * 128]
                        rhs = xg_t[:, k * cap:(k + 1) * cap]
                        nc.tensor.matmul(p3[:], lhs, rhs, start=(k == 0), stop=(k == KW - 1))
                    a = work.tile([128, cap], DT, tag="act_a")
                    nc.scalar.activation(a[:], p1[:], LR, bias=b_ap(boff + m), alpha=0.01)
                    t3 = work.tile([128, cap], DT, tag="act_b")
                    nc.vector.tensor_scalar_add(t3[:], p3[:], b_ap(boff + MI + m))
                    ht = hts.tile([128, cap], DT, tag="ht", bufs=LAG + 3)
                    nc.gpsimd.tensor_mul(ht[:], a[:], t3[:])
                    pend.append((m, ht))
                    if len(pend) > LAG:
                        stage2(*pend.pop(0))
                    if m != MI - 1:
                        yield
                for args in pend:
                    stage2(*args)
                yo = outs.tile([128, cap], DT, tag="yo")
                nc.vector.tensor_copy(yo[:], py[:])
                off = 0 if slot == 0 else cap0
                nc.sync.dma_start(yr[:, off:off + cap], yo[:])
                yield

            def shared_steps():
                """Generator: one step per (chunk, m-tile); stage-2 flush of
                chunk c happens during chunk c+1 (full-chunk lag)."""
                pend = []

                def flush(pz, c, hs_list):
                    for m, hs in hs_list:
                        lhs = swp_t[:, m * MBLK + KW * 256:m * MBLK + KW * 256 + OUT]
                        nc.tensor.matmul(pz[:], lhs, hs[:], start=(m == 0), stop=(m == MS - 1))
                    zo = outs.tile([128, CH], DT, tag="zo")
                    nc.vector.tensor_copy(zo[:], pz[:])
                    eng = nc.sync if c % 2 == 0 else nc.scalar
                    eng.dma_start(zt[:, c * CH:(c + 1) * CH], zo[:])

                for c in range(NCH):
                    pz = ps.tile([128, CH], F32, tag="pz", bufs=1)
                    hs_list = []
                    for m in range(MS):
                        mb = m * MBLK
                        p1 = ps.tile([128, CH], F32, tag="p1", bufs=2)
                        p3 = ps.tile([128, CH], F32, tag="p3", bufs=2)
                        for k in range(KW):
                            lhs = swp_t[:, mb + k * 128:mb + (k + 1) * 128]
                            rhs = xt_ts[c][:, k * CH:(k + 1) * CH]
                            nc.tensor.matmul(p1[:], lhs, rhs, start=(k == 0), stop=(k == KW - 1))
                        for k in range(KW):
                            lhs = swp_t[:, mb + KW * 128 + k * 128:mb + KW * 128 + (k + 1) * 128]
                            rhs = xt_ts[c][:, k * CH:(k + 1) * CH]
                            nc.tensor.matmul(p3[:], lhs, rhs, start=(k == 0), stop=(k == KW - 1))
                        a = work.tile([128, CH], DT, tag="act_a")
                        nc.scalar.activation(a[:], p1[:], LR,
                                             bias=b_ap(4 * MI + m), alpha=0.01)
                        t3 = work.tile([128, CH], DT, tag="act_b")
                        nc.vector.tensor_scalar_add(t3[:], p3[:], b_ap(4 * MI + MS + m))
                        hs = hts.tile([128, CH], DT, tag="hs", bufs=MS + 2)
                        nc.gpsimd.tensor_mul(hs[:], a[:], t3[:])
                        hs_list.append((m, hs))
                        if not (c == NCH - 1 and m == MS - 1):
                            yield
                    pend.append((pz, c, hs_list))
                    if len(pend) > 1:
                        flush(*pend.pop(0))
                yield
                for args in pend:
                    flush(*args)
                yield

            # ---- interleave: A/B = expert slot m-tile, S = shared m-tile.
            # Default tuned to the DMA arrival schedule (SP ~0.58MB/group,
            # ACT xga+swp+xt chunks); override via K_PATTERN for experiments.
            pattern = os.environ.get(
                "K_PATTERN",
                "SSSS AA SS AA SS AA SS AA SS BB SS BB SS BB S BB")
            gens = {"A": expert_slot(0), "B": expert_slot(1), "S": shared_steps()}
            for ch in pattern:
                if ch == " ":
                    continue
                g = gens.get(ch)
                if g is None:
                    continue
                try:
                    next(g)
                except StopIteration:
                    gens[ch] = None
            for g in gens.values():
                if g is None:
                    continue
                for _ in g:
                    pass

    if legalize:
        _legalize_waits(nc)
    return nc


_NC_CACHE = {}


def _kblocks(mat, nk):
    """[nk*128, F] -> [128, nk*F], col block k = mat[128k:128(k+1), :]."""
    f = mat.shape[1]
    return mat.reshape(nk, 128, f).transpose(1, 0, 2).reshape(128, -1)


def _pack_mblocks(w1e, w3e, w2oe, nm):
    """Consumption-ordered weight pack: nm m-blocks of
    [w1 k-blocks (KW*128) | w3 k-blocks | w2ot (OUT)] = MBLK cols each."""
    # w1e/w3e: [nm*128, W];  w2oe: [nm*128, OUT]
    w1b = w1e.T.reshape(KW, 128, nm, 128)   # [k, kr, m, mc]
    w3b = w3e.T.reshape(KW, 128, nm, 128)
    w2b = w2oe.reshape(nm, 128, OUT)        # [m, mr, OUT]
    cols = []
    for m in range(nm):
        for k in range(KW):
            cols.append(w1b[k, :, m, :])
        for k in range(KW):
            cols.append(w3b[k, :, m, :])
        cols.append(w2b[m])
    return np.concatenate(cols, axis=1).astype(NPDT)


def prepare(x, task_id, gate_w, w1, b1, w2, b2, w3, b3,
            sw1, sb1, sw2, sb2, sw3, sb3, ow, ob):
    """Host-side routing + packing."""
    x = np.asarray(x, np.float32)
    f32 = lambda a: np.asarray(a, np.float32)
    gate_w, w1, b1, w2, b2, w3, b3 = map(f32, (gate_w, w1, b1, w2, b2, w3, b3))
    sw1, sb1, sw2, sb2, sw3, sb3, ow, ob = map(f32, (sw1, sb1, sw2, sb2, sw3, sb3, ow, ob))

    # ---- host gate: softmax + top-2 ----
    logits = x @ gate_w.T
    logits -= logits.max(axis=1, keepdims=True)
    ex = np.exp(logits)
    scores = ex / ex.sum(axis=1, keepdims=True)
    order = np.argsort(-scores, axis=1, kind="stable")[:, :TOPK]

    tok_lists = [np.nonzero((order == e).any(axis=1))[0] for e in range(E)]
    counts = np.array([len(t) for t in tok_lists])
    rank = np.argsort(-counts, kind="stable")
    slotA = [int(rank[i]) for i in range(NCORES)]          # big experts
    slotB = [int(rank[E - 1 - i]) for i in range(NCORES)]  # small experts
    r16 = lambda n: max(64, -(-n // 16) * 16)
    cap0 = r16(max(counts[e] for e in slotA))
    cap1 = r16(max(counts[e] for e in slotB))

    key = (cap0, cap1)
    if key not in _NC_CACHE:
        _NC_CACHE[key] = _build_nc(cap0, cap1)
    nc = _NC_CACHE[key]

    # ---- shared-expert packing (same for every core except slice) ----
    xtp = x.T.reshape(KW, 128, 8, 256).transpose(1, 2, 0, 3).reshape(128, -1)
    xtp = np.ascontiguousarray(xtp).astype(NPDT)   # chunk-major, k inside

    in_maps = []
    for c in range(NCORES):
        eA, eB = slotA[c], slotB[c]
        s = slice(c * SHS, (c + 1) * SHS)

        def gather(e, cap):
            toks = tok_lists[e]
            xg = np.zeros((W, cap), np.float32)
            xg[:, :len(toks)] = x[toks].T
            return _kblocks(xg, KW).astype(NPDT)

        swp = _pack_mblocks(sw1[s], sw3[s], sw2[:, s].T @ ow.T, MS)

        bias_cols = [b1[eA].reshape(MI, 128).T, b3[eA].reshape(MI, 128).T,
                     b1[eB].reshape(MI, 128).T, b3[eB].reshape(MI, 128).T,
                     sb1[s].reshape(MS, 128).T, sb3[s].reshape(MS, 128).T]

        in_maps.append({
            "xga": gather(eA, cap0),
            "xgb": gather(eB, cap1),
            "wga": _pack_mblocks(w1[eA], w3[eA], w2[eA].T @ ow.T, MI),
            "wgb": _pack_mblocks(w1[eB], w3[eB], w2[eB].T @ ow.T, MI),
            "swp": swp,
            "xt": xtp,
            "bias": np.ascontiguousarray(np.concatenate(bias_cols, axis=1)),
        })

    combine_w = np.zeros((B, E), np.float32)
    rows = np.arange(B)
    combine_w[rows[:, None], order] = np.take_along_axis(scores, order, axis=1)
    base = combine_w @ (b2 @ ow.T) + sb2 @ ow.T + ob

    return dict(nc=nc, in_maps=in_maps, cap0=cap0, cap1=cap1,
                slotA=slotA, slotB=slotB, tok_lists=tok_lists,
                combine_w=combine_w, base=base)


def combine(p, results):
    """Combine per-core device partials into the full [B, OUT] output."""
    cap0, tok_lists, combine_w = p["cap0"], p["tok_lists"], p["combine_w"]
    out = p["base"].astype(np.float32).copy()
    for c in range(NCORES):
        r = results[c]
        out += r["zt"].astype(np.float32).T
        for slot, e in ((0, p["slotA"][c]), (1, p["slotB"][c])):
            toks = tok_lists[e]
            off = 0 if slot == 0 else cap0
            yre = r["yr"][:, off:off + len(toks)].astype(np.float32)
            out[toks] += combine_w[toks, e][:, None] * yre.T
    return out


def kernel(x, task_id, gate_w, w1, b1, w2, b2, w3, b3,
           sw1, sb1, sw2, sb2, sw3, sb3, ow, ob):
    global LAST_RESULTS
    p = prepare(x, task_id, gate_w, w1, b1, w2, b2, w3, b3,
                sw1, sb1, sw2, sb2, sw3, sb3, ow, ob)
    res = run_bass_kernel_spmd(
        p["nc"], p["in_maps"], core_ids=list(range(NCORES)),
        trace=TRACE, **TRACE_KW)
    LAST_RESULTS = res
    return combine(p, res.results)
